# revision 25
# baseline (speedup 1.0000x reference)
"""Trainium2 kernel for nn_LinearDynamics: chunked two-level scan, 8-core data parallel.

v10 final (57667ns measured; v3 baseline 60344ns): all-bf16 matmul path
    (fp8 u) with a tuned front-end and tail. Measured facts this rests on:
    - HAM needs ~3.4us of CONTINUOUS PE busy to clock up 1.2->2.4GHz; idle
      gaps reset the window. Warmup (14 N=128 + 4 N=512 matmuls) bridges
      the whole input-DMA wait so HAM flips exactly once, at phase A start.
    - the 4 N=512 zero-matmuls pre-fill the psC state banks (zeros with
      has_written set), letting phase B' ACT-copy its d=0 identity term
      instead of spending 4 PE matmuls.
    - input DMA is aggregate-bandwidth-bound (~0.25-0.3 MB/us early) and
      each desc costs ~0.65us of queue time: few, large descs in need-order.
    - tail output DMAs split across both HWDGE queues, each step's descs
      emitted after that step's copies (measured best of 4 tail layouts).
    Dead ends measured: fp8 DoubleRow streams 2N cols at 1 col/cycle (no
    win, 2x worse cold); psum-escape floor (~690ns per [128,512], DVE/ACT
    only) kills any phase C restructure that adds a per-step cast; chip
    downclocks ~16-20%% under sustained load (P0) - compare runs by trace
    structure, not raw exec time.

Per core (128 batch rows, state transposed xT [d_x=128, b=128]):
  x_{t+1} = x_t + (x_t @ dtA + u_t @ B2),  dtA = dt*A, B2 = dt*B, M = I + dtA

Chunks: S=16 chunks of L=16 steps, grouped NG=4 x GS=4 (GW=512 cols).
Host precomputes (float64, cast bf16):
  Wt slot i (i<8): rows 0:64 = N_{15-2i}, rows 64:128 = N_{14-2i}, N_p = B2@M^p
  Wt slot 8/9: B2 zero-padded to rows 0:64 / rows 64:128; slot 10: dtA
  MP_d = M^(d*L)  d=0..4

Phase A: W_s = sum_j u_{sL+j} @ N_{15-j}; u pair-packed on partitions,
  8 matmuls/group into PSUM; ACT copies psum -> Wext (bf16).
Phase B': boundary X-block(g) into psC[g]: d=0 term ACT-copied into the
  pre-filled psC bank, 3 windowed W-term matmuls accumulate on top (+ for
  g>0 one chain matmul X-block(g-1) @ M^{4L}). Group accumulation left OPEN.
Phase C: PE keeps accumulating into the same psum bank:
    psC[g] += u_j @ B2pad + xr_{k-1} @ dtA     (psum IS the f32 state)
  one copy per step (DVE/ACT alternate) -> bf16 xr tile = next matmul
  input AND DMA output.

Host-sim rel err ~3.8e-3.
"""

import ml_dtypes
import numpy as np

DT = 0.1
BATCH, T, DX, DU = 1024, 256, 128, 64
NCORES = 8
BPC = BATCH // NCORES  # 128
S, L = 16, 16
NG, GS = 4, 4
GW = GS * BPC  # 512

_CACHE = {}


def _build(debug=False):
    import concourse.mybir as mybir
    import concourse.tile as tile
    from concourse import bacc

    f32 = mybir.dt.float32
    bf16 = mybir.dt.bfloat16
    fp8 = mybir.dt.float8e4

    nc = bacc.Bacc("TRN2", target_bir_lowering=False, debug=debug)
    wt_d = nc.declare_dram_parameter("WT", [DX, 11 * DX], bf16, isOutput=False)
    mp_d = nc.declare_dram_parameter("MP", [DX, 5 * DX], bf16, isOutput=False)
    w0_d = nc.declare_dram_parameter("W0T", [DX, 4 * DX], bf16, isOutput=False)
    u_d = nc.declare_dram_parameter("uT", [NG, DX, 8 * GW], fp8, isOutput=False)
    y_d = nc.declare_dram_parameter("yT", [NG, 8, DX, 2 * GW], bf16, isOutput=True)

    with tile.TileContext(nc) as tc:
        with (
            tc.tile_pool(name="cw", bufs=1) as cw,
            tc.tile_pool(name="psA", bufs=2, space="PSUM") as psA,
            tc.tile_pool(name="psW", bufs=1, space="PSUM") as psW,
            tc.tile_pool(name="psC", bufs=1, space="PSUM") as psC,
        ):
            # Input DMAs, priority-ordered: weights on the ACT queue, u on
            # the SYNC queue. First slices sized so phase A can start ~8.8us
            # and stream with DMA arrival (~0.3 MB/us aggregate input bw).
            Wt = cw.tile([DX, 11 * DX], bf16)
            MP = cw.tile([DX, 5 * DX], bf16)
            Wext = cw.tile([DX, (4 + S) * DX], bf16)
            # scalar queue order: phase A slots first, then phase B's MP/W0T
            # (needed ~14us), then the phase C slots (needed only ~20us).
            u_sb = cw.tile([DX, NG * 8 * GW], fp8)

            def udma(eng, g, b0, b1):  # u pair-blocks [b0,b1) of group g
                c0, c1 = (g * 8 + b0) * GW, (g * 8 + b1) * GW
                eng.dma_start(u_sb[:, c0:c1], u_d[g][:, b0 * GW : b1 * GW])

            # u is split across BOTH HWDGE rings so the two descriptor
            # queues deliver it in parallel (one ring alone runs at ~half
            # the aggregate input bandwidth while the other moves weights):
            # sync carries g0 + g2, scalar interleaves g1 + g3 between the
            # weight loads at their need-times.
            nc.scalar.dma_start(Wt[:, 0 : 4 * DX], wt_d[:, 0 : 4 * DX])
            nc.scalar.dma_start(Wt[:, 4 * DX : 8 * DX], wt_d[:, 4 * DX : 8 * DX])
            udma(nc.scalar, 1, 0, 8)
            nc.scalar.dma_start(MP[:], mp_d[:])
            nc.scalar.dma_start(Wext[:, 0 : 4 * DX], w0_d[:])
            udma(nc.scalar, 3, 0, 8)
            nc.scalar.dma_start(Wt[:, 8 * DX :], wt_d[:, 8 * DX :])
            udma(nc.sync, 0, 0, 4)
            udma(nc.sync, 0, 4, 8)
            udma(nc.sync, 2, 0, 8)

            xr = [
                cw.tile([DX, (L + 1) * GW], bf16, name=f"xr{g}") for g in range(NG)
            ]
            psCt = [psC.tile([DX, GW], f32, name=f"psCt{g}") for g in range(NG)]

            # PE warm-up: CONTINUOUS matmul activity from ~7.1us until the
            # first u/Wt slices land (~10.3us) so HAM flips to 8/8 exactly
            # once, at phase A start (idle gaps reset the 4096-cycle busy
            # window, so the warmup must bridge the whole DMA wait). The 4
            # N=512 zero-matmuls pre-fill the psC state banks (zeros,
            # has_written set) for phase B's d=0 ACT copy.
            scr = cw.tile([DX, GW], bf16)
            nc.gpsimd.memset(scr[:], 0)
            psw = psW.tile([DX, DX], f32)
            for _ in range(10):
                nc.tensor.matmul(
                    psw[:], scr[:, 0:DX], scr[:, 0:DX], start=True, stop=True
                )
            for g in range(NG):
                nc.tensor.matmul(
                    psCt[g][:], scr[:, 0:DX], scr[:], start=True, stop=True
                )

            def ccopy(idx, dst, src):
                # alternate DVE / ACT for the per-step psum->bf16 copy
                if idx % 2 == 0:
                    nc.vector.tensor_copy(dst, src)
                else:
                    nc.scalar.copy(dst, src)

            for g in range(NG):
                # phase A: W for the 4 chunks of group g
                ps = psA.tile([DX, GW], f32)
                for i in range(8):
                    nc.tensor.matmul(
                        ps[:],
                        Wt[:, i * DX : (i + 1) * DX],
                        u_sb[:, g * 8 * GW + i * GW : g * 8 * GW + (i + 1) * GW],
                        start=(i == 0),
                        stop=(i == 7),
                    )
                nc.scalar.copy(
                    Wext[:, (4 + g * GS) * DX : (4 + (g + 1) * GS) * DX], ps[:]
                )
                # phase B': d=0 identity term is an ACT copy into the
                # pre-filled psC bank; d=1..3 matmuls accumulate on top
                # (+ chain term for g>0); group accumulation left OPEN.
                sc0 = (4 * g + 3) * DX
                nc.scalar.copy(psCt[g][:], Wext[:, sc0 : sc0 + GW])
                for d in range(1, 4):
                    sc = (4 * g + 3 - d) * DX
                    nc.tensor.matmul(
                        psCt[g][:],
                        MP[:, d * DX : (d + 1) * DX],
                        Wext[:, sc : sc + GW],
                        start=False,
                        stop=False,
                        skip_group_check=True,
                    )
                if g > 0:
                    nc.tensor.matmul(
                        psCt[g][:],
                        MP[:, 4 * DX : 5 * DX],
                        xr[g - 1][:, 0:GW],
                        start=False,
                        stop=False,
                    )
                ccopy(g, xr[g][:, 0:GW], psCt[g][:])

            # phase C: psum IS the state; one copy per step per group
            ci = 0
            for k in range(1, L + 1):
                j = k - 1
                par = j & 1
                i = j >> 1
                for g in range(NG):
                    nc.tensor.matmul(
                        psCt[g][:],
                        Wt[:, (8 + par) * DX : (9 + par) * DX],
                        u_sb[:, g * 8 * GW + i * GW : g * 8 * GW + (i + 1) * GW],
                        start=False,
                        stop=False,
                    )
                    nc.tensor.matmul(
                        psCt[g][:],
                        Wt[:, 10 * DX : 11 * DX],
                        xr[g][:, (k - 1) * GW : k * GW],
                        start=False,
                        stop=(k == L),
                    )
                    ccopy(ci, xr[g][:, k * GW : (k + 1) * GW], psCt[g][:])
                    ci += 1
                    # 2-step output slabs on sync for k<=12
                    if k % 2 == 0 and k <= 12:
                        m = k // 2 - 1
                        nc.sync.dma_start(
                            y_d[g][m],
                            xr[g][:, (2 * m + 1) * GW : (2 * m + 3) * GW],
                        )
                # Tail descs (k=14 slab, k=15/16 singles) go AFTER all four
                # copies of that step, split across both queues, so a desc
                # waiting on one group's copy never blocks another group's
                # copy on the same engine queue.
                if k == 14:
                    for g in range(NG):
                        eng = [nc.sync, nc.sync, nc.scalar, nc.scalar][g]
                        eng.dma_start(y_d[g][6], xr[g][:, 13 * GW : 15 * GW])
                elif k == 15:
                    for g in range(NG):
                        eng = [nc.scalar, nc.scalar, nc.sync, nc.sync][g]
                        eng.dma_start(
                            y_d[g][7][:, 0:GW], xr[g][:, 15 * GW : 16 * GW]
                        )
                elif k == 16:
                    for g in range(NG):
                        eng = [nc.sync, nc.sync, nc.scalar, nc.scalar][g]
                        eng.dma_start(
                            y_d[g][7][:, GW : 2 * GW], xr[g][:, 16 * GW : 17 * GW]
                        )
    nc.compile()
    return nc


def _get_nc():
    if "nc" not in _CACHE:
        _CACHE["nc"] = _build()
    return _CACHE["nc"]


def _host_mats(A, Bmat):
    M64 = np.eye(DX, dtype=np.float64) + DT * A.astype(np.float64)
    B264 = DT * Bmat.astype(np.float64)
    Np = []
    Mp = np.eye(DX, dtype=np.float64)
    for p in range(L):
        Np.append((B264 @ Mp).astype(np.float32))
        Mp = Mp @ M64
    ML64 = Mp  # M^L
    Wt = np.zeros((DX, 11 * DX), dtype=np.float32)
    for i in range(8):
        Wt[0:DU, i * DX : (i + 1) * DX] = Np[15 - 2 * i]
        Wt[DU : 2 * DU, i * DX : (i + 1) * DX] = Np[14 - 2 * i]
    B2 = B264.astype(np.float32)
    Wt[0:DU, 8 * DX : 9 * DX] = B2
    Wt[DU : 2 * DU, 9 * DX : 10 * DX] = B2
    Wt[:, 10 * DX : 11 * DX] = (DT * A.astype(np.float64)).astype(np.float32)
    MP = np.zeros((DX, 5 * DX), dtype=np.float32)
    Md = np.eye(DX, dtype=np.float64)
    for d in range(5):
        MP[:, d * DX : (d + 1) * DX] = Md.astype(np.float32)
        Md = Md @ ML64
    return Wt.astype(ml_dtypes.bfloat16), MP.astype(ml_dtypes.bfloat16)


def _prep_inputs(initial_state, u_traj, A, Bmat):
    Wt, MP = _host_mats(A, Bmat)
    in_maps = []
    for c in range(NCORES):
        rc = slice(c * BPC, (c + 1) * BPC)
        w0 = np.zeros((DX, 4 * DX), dtype=np.float32)
        w0[:, 3 * DX :] = initial_state[rc].T
        uc = u_traj[rc]  # [b, t, du]; t = (4g+q)*16 + 2i+par
        ut = uc.reshape(BPC, NG, GS, 8, 2, DU)  # b, g, q, i, par, du
        ut = ut.transpose(1, 4, 5, 3, 2, 0)  # g, par, du, i, q, b
        uT = (
            np.ascontiguousarray(ut)
            .reshape(NG, DX, 8 * GW)
            .astype(ml_dtypes.float8_e4m3)
        )
        in_maps.append(
            {
                "WT": Wt,
                "MP": MP,
                "W0T": w0.astype(ml_dtypes.bfloat16),
                "uT": uT,
            }
        )
    return in_maps


def _assemble(results, initial_state):
    out = np.empty((BATCH, T + 1, DX), dtype=np.float32)
    out[:, 0, :] = initial_state
    for c in range(NCORES):
        rc = slice(c * BPC, (c + 1) * BPC)
        yT = results[c]["yT"]  # [g, m, dx, kin*q*b] bf16
        y = np.asarray(yT).reshape(NG, 8, DX, 2, GS, BPC)  # g, m, dx, kin, q, b
        y = y.transpose(5, 0, 4, 1, 3, 2)  # b, g, q, m, kin, dx
        out[rc, 1:, :] = y.reshape(BPC, T, DX).astype(np.float32)
    return out


def run(initial_state, u_traj, A, Bmat, trace=False, **trace_kwargs):
    from concourse.bass_utils import run_bass_kernel_spmd

    nc = _get_nc()
    in_maps = _prep_inputs(initial_state, u_traj, A, Bmat)
    res = run_bass_kernel_spmd(
        nc, in_maps, list(range(NCORES)), trace=trace, **trace_kwargs
    )
    out = _assemble(res.results, initial_state)
    return out, res


def kernel(initial_state, u_traj, A, Bmat):
    out, _ = run(initial_state, u_traj, A, Bmat)
    return out
